# revision 4
# baseline (speedup 1.0000x reference)
"""BQuantConv1d Trainium2 kernel.

Math: the reference's 256-entry LUT gather per (token, group, out-feature) is
algebraically out = X @ W' + bias with a single dense weight matrix
    W'[i, f] = sum_k scale[k, f] * sgn(bit_{7-(i%8)}(binary[0, k, i//8, f]))
(the per-plane scale is a per-output-column factor, so the 8 sign-matmuls of
the bit planes collapse into one matmul once scale is folded into the weights
on the host — the same host-side combine the plane-sharded variant did after
the fact, just moved before the matmul).

Device program per core (output-feature sharding, 96 features per core):
  - packed input [128, 2112] bf16 per execution: x^T in 6 K-tiles of 128
    input features ([128, 6*256]) followed by this core's W' column slice in
    the matching K-tile layout ([128, 6*96]),
  - 6 PSUM-accumulated matmuls (stationary = W' tile [128, 96], streaming =
    x^T tile [128, 256]) producing out[f, b] = [96, 256] f32,
  - PSUM -> SBUF copy on DVE, f32 output slice [96, 256].
X is replicated across cores; W'/out are column-sharded. The host transposes/
concatenates the 8 slices and adds bias.

Timing structure: the For_i hardware loop carries an all-engine barrier at
the back edge, so UNROLL bodies are emitted per loop iteration (n_iter total
body executions). DMA fixed cost (~2us completion stall per dma_start,
serialized per HWDGE ring) dominates small transfers, so input DMAs are
batched 4 bodies at a time (QUAD) and issued alternately on the two HWDGE
rings (sync/scalar), and all UNROLL output slices go out in one DMA.
"""

import numpy as np
import ml_dtypes

B = 256          # flattened tokens 4*64
NX = 768         # input features
NF = 768         # output features
NCORES = 8
BITS = 8         # kept for compatibility (== NCORES)
FS = NF // NCORES  # 96 output features per core
KT = 6           # contraction tiles of 128
XW = KT * B      # 1536 columns of x^T
COLS = XW + KT * FS  # 2112 packed input columns
QUAD = 4         # bodies per input DMA
UNROLL = 8       # bodies per hardware-loop iteration

_CACHE = {}


def _emit_iter(nc, tc, bass, mybir, pools, inp_d, out_d, mode="full", unroll=UNROLL):
    fp32 = mybir.dt.float32
    bf16 = mybir.dt.bfloat16
    const, opool, psum = pools

    if mode == "empty":
        zz = const.tile([128, 1], fp32, tag="zz", name="zz")
        nc.gpsimd.memset(zz[:], 0.0)
        return

    nquad = (unroll + QUAD - 1) // QUAD
    quads = []
    for q in range(nquad):
        inp = const.tile([128, QUAD * COLS], bf16, tag=f"inp{q % 2}", name="inp")
        eng = nc.sync if q % 2 == 0 else nc.scalar
        eng.dma_start(inp[:], inp_d.ap())
        quads.append(inp)
    if mode == "dma":
        return

    out_sb = opool.tile([128, unroll * B], fp32, tag="out", name="out_sb")
    for u in range(unroll):
        inp = quads[u // QUAD]
        base = (u % QUAD) * COLS
        pm = psum.tile([128, B], fp32, tag="pm", name="pm")
        for t in range(KT):
            nc.tensor.matmul(
                pm[0:FS, :],
                inp[:, base + XW + t * FS : base + XW + (t + 1) * FS],
                inp[:, base + t * B : base + (t + 1) * B],
                start=(t == 0),
                stop=(t == KT - 1),
            )
        nc.vector.tensor_scalar(
            out_sb[0:FS, u * B : (u + 1) * B], pm[0:FS, :], 0.0, None,
            mybir.AluOpType.add,
        )
    nc.gpsimd.dma_start(out_d.ap()[:, 0 : unroll * B], out_sb[0:FS, 0 : unroll * B])


def _declare_io(nc, mybir, unroll=UNROLL):
    fp32 = mybir.dt.float32
    bf16 = mybir.dt.bfloat16
    # packed: cols [0, XW) = x^T K-tiles, cols [XW, COLS) = W' column slice,
    # replicated QUAD times (each execution streams its own copy from HBM)
    inp_d = nc.dram_tensor("inp", [128, QUAD * COLS], bf16, kind="ExternalInput")
    # out[f_local, u*B + b] — final output slice (pre-bias) of execution u
    out_d = nc.dram_tensor("out", [FS, max(unroll, 1) * B], fp32,
                           kind="ExternalOutput")
    return inp_d, out_d


def _build_program(n_iter=1, mode="full", unroll=UNROLL):
    import concourse.bass as bass
    import concourse.tile as tile
    from concourse import bacc, mybir

    nc = bacc.Bacc("TRN2", target_bir_lowering=False, debug=False)
    io = _declare_io(nc, mybir, unroll=(unroll if n_iter > 1 else 1))

    with tile.TileContext(nc) as tc:
        with (
            tc.tile_pool(name="const", bufs=2) as const,
            tc.tile_pool(name="opool", bufs=2) as opool,
            tc.tile_pool(name="psum", bufs=2, space=bass.MemorySpace.PSUM) as psum,
        ):
            pools = (const, opool, psum)
            if n_iter == 1:
                _emit_iter(nc, tc, bass, mybir, pools, *io, mode=mode, unroll=1)
            else:
                assert n_iter % unroll == 0, (n_iter, unroll)
                with tc.For_i(0, n_iter // unroll, 1):
                    _emit_iter(nc, tc, bass, mybir, pools, *io, mode=mode,
                               unroll=unroll)

    nc.compile()
    return nc


def _prep_inputs(x, binary, scale):
    xf = np.asarray(x, dtype=np.float32).reshape(B, NX)
    # xT[p, t*B + b] = xf[b, t*128 + p]
    xT = np.ascontiguousarray(
        xf.T.reshape(KT, 128, B).transpose(1, 0, 2)
    ).reshape(128, XW).astype(ml_dtypes.bfloat16)

    bins = np.asarray(binary)[0].astype(np.uint8)        # [8, 96, 768]
    bits = np.unpackbits(bins[:, :, :, None], axis=3)    # [..., p] = bit (7-p)
    sgn = bits.astype(np.float32) * 2.0 - 1.0            # [8k, 96m, 768f, 8p]
    sc = np.asarray(scale, dtype=np.float32)[0]          # [8, 768]
    W = np.einsum("kmfp,kf->mpf", sgn, sc).reshape(NX, NF)
    Wr = W.reshape(KT, 128, NCORES, FS)                  # [t, p, j, f]

    in_maps = []
    for j in range(NCORES):
        wj = np.ascontiguousarray(Wr[:, :, j, :].transpose(1, 0, 2)).reshape(
            128, KT * FS
        ).astype(ml_dtypes.bfloat16)
        inp = np.ascontiguousarray(
            np.tile(np.concatenate([xT, wj], axis=1), (1, QUAD))
        )
        in_maps.append({"inp": inp})
    return in_maps


def kernel(x, scale, binary, bias, _trace=False):
    from concourse.bass_utils import run_bass_kernel_spmd

    if "nc" not in _CACHE:
        _CACHE["nc"] = _build_program()
    nc = _CACHE["nc"]

    in_maps = _prep_inputs(x, binary, scale)
    res = run_bass_kernel_spmd(nc, in_maps, core_ids=list(range(NCORES)), trace=_trace)
    _CACHE["last_result"] = res

    outT = np.concatenate(
        [np.asarray(res.results[j]["out"])[:, 0:B] for j in range(NCORES)], axis=0
    )  # [768, 256]
    out = outT.T + np.asarray(bias, dtype=np.float32)[None, :]
    return out.reshape(4, 64, NF).astype(np.float32)


# revision 5
# speedup vs baseline: 8.9378x; 8.9378x over previous
"""BQuantConv1d Trainium2 kernel.

Math: the reference's 256-entry LUT gather per (token, group, out-feature) is
algebraically out = X @ W' + bias with a single dense weight matrix
    W'[i, f] = sum_k scale[k, f] * sgn(bit_{7-(i%8)}(binary[0, k, i//8, f]))
(the per-plane scale is a per-output-column factor, so the 8 sign-matmuls of
the bit planes collapse into one matmul once scale is folded into the weights
on the host — the same host-side combine the plane-sharded variant did after
the fact, just moved before the matmul).

Device program per core (output-feature sharding, 96 features per core), per
kernel execution ("body"):
  - x^T in 6 K-tiles of 128 input features: [128, 6*256] fp8e3 (e3m4 — x is
    ~N(0,1), well inside e3m4 range; quantization error ~0.5% of output),
  - this core's W' column slice in matching K-tile layout [128, 6*96] bf16,
  - 6 PSUM-accumulated matmuls (stationary = W' tile [128, 96] bf16,
    streaming = x^T tile [128, 256] fp8) producing out[f, b] = [96, 256] f32,
  - PSUM -> SBUF copy on DVE, output slice [96, 256].
X is replicated across cores; W'/out are column-sharded. The host transposes/
concatenates the 8 slices and adds bias.

Timing structure: the For_i hardware loop carries an all-engine barrier at
the back edge, so UNROLL bodies are emitted per loop iteration (n_iter total
body executions). DMA fixed cost (~2us completion stall per dma_start,
serialized per HWDGE ring) dominates small transfers, so inputs are streamed
as GROUP-body mega-DMAs alternated across the two HWDGE rings (x and w of a
group on opposite rings), and outputs go out one DMA per group on the SWDGE
(gpsimd) queue.
"""

import numpy as np
import ml_dtypes

B = 256          # flattened tokens 4*64
NX = 768         # input features
NF = 768         # output features
NCORES = 8
BITS = 8         # kept for compatibility (== NCORES)
FS = NF // NCORES  # 96 output features per core
KT = 6           # contraction tiles of 128
XW = KT * B      # 1536 x^T columns per body
WC = KT * FS     # 576 w columns per body
GROUP = 8        # bodies per input DMA
UNROLL = 16      # bodies per hardware-loop iteration

X_NP = ml_dtypes.float8_e3m4
OUT_F32 = True   # device output dtype (False -> bf16)

_CACHE = {}


def _emit_iter(nc, tc, bass, mybir, pools, x_d, w_d, out_d, mode="full",
               unroll=UNROLL):
    fp32 = mybir.dt.float32
    bf16 = mybir.dt.bfloat16
    x_dt = mybir.dt.float8e3
    out_dt = fp32 if OUT_F32 else bf16
    const, opool, psum = pools

    if mode == "empty":
        zz = const.tile([128, 1], fp32, tag="zz", name="zz")
        nc.gpsimd.memset(zz[:], 0.0)
        return

    ngr = (unroll + GROUP - 1) // GROUP
    xs, ws = [], []
    for g in range(ngr):
        nb = min(GROUP, unroll - g * GROUP)
        xm = const.tile([128, GROUP * XW], x_dt, tag=f"x{g % 2}", name="xm")
        wm = const.tile([128, GROUP * WC], bf16, tag=f"w{g % 2}", name="wm")
        ring0, ring1 = (nc.sync, nc.scalar) if g % 2 == 0 else (nc.scalar, nc.sync)
        ring0.dma_start(xm[:, 0 : nb * XW], x_d.ap()[:, 0 : nb * XW])
        ring1.dma_start(wm[:, 0 : nb * WC], w_d.ap()[:, 0 : nb * WC])
        xs.append(xm)
        ws.append(wm)
    if mode == "dma":
        return

    out_sb = opool.tile([128, unroll * B], out_dt, tag="out", name="out_sb")
    for u in range(unroll):
        g, s = u // GROUP, u % GROUP
        xm, wm = xs[g], ws[g]
        pm = psum.tile([128, B], fp32, tag="pm", name="pm")
        for t in range(KT):
            nc.tensor.matmul(
                pm[0:FS, :],
                wm[:, s * WC + t * FS : s * WC + (t + 1) * FS],
                xm[:, s * XW + t * B : s * XW + (t + 1) * B],
                start=(t == 0),
                stop=(t == KT - 1),
            )
        nc.vector.tensor_scalar(
            out_sb[0:FS, u * B : (u + 1) * B], pm[0:FS, :], 0.0, None,
            mybir.AluOpType.add,
        )
        if s == GROUP - 1 or u == unroll - 1:
            lo = g * GROUP * B
            hi = (u + 1) * B
            nc.gpsimd.dma_start(
                out_d.ap()[:, lo:hi], out_sb[0:FS, lo:hi]
            )


def _declare_io(nc, mybir, unroll=UNROLL):
    fp32 = mybir.dt.float32
    bf16 = mybir.dt.bfloat16
    out_dt = fp32 if OUT_F32 else bf16
    # x^T K-tile stream, one copy per body in the group (each execution
    # streams its own input from HBM); w likewise
    x_d = nc.dram_tensor("x", [128, GROUP * XW], mybir.dt.float8e3,
                         kind="ExternalInput")
    w_d = nc.dram_tensor("w", [128, GROUP * WC], bf16, kind="ExternalInput")
    # out[f_local, u*B + b] — final output slice (pre-bias) of execution u
    out_d = nc.dram_tensor("out", [FS, max(unroll, 1) * B], out_dt,
                           kind="ExternalOutput")
    return x_d, w_d, out_d


def _build_program(n_iter=1, mode="full", unroll=UNROLL):
    import concourse.bass as bass
    import concourse.tile as tile
    from concourse import bacc, mybir

    nc = bacc.Bacc("TRN2", target_bir_lowering=False, debug=False)
    io = _declare_io(nc, mybir, unroll=(unroll if n_iter > 1 else 1))

    with tile.TileContext(nc) as tc:
        with (
            tc.tile_pool(name="const", bufs=2) as const,
            tc.tile_pool(name="opool", bufs=2) as opool,
            tc.tile_pool(name="psum", bufs=2, space=bass.MemorySpace.PSUM) as psum,
        ):
            pools = (const, opool, psum)
            if n_iter == 1:
                _emit_iter(nc, tc, bass, mybir, pools, *io, mode=mode, unroll=1)
            else:
                assert n_iter % unroll == 0, (n_iter, unroll)
                with tc.For_i(0, n_iter // unroll, 1):
                    _emit_iter(nc, tc, bass, mybir, pools, *io, mode=mode,
                               unroll=unroll)

    nc.compile()
    return nc


def _prep_inputs(x, binary, scale):
    xf = np.asarray(x, dtype=np.float32).reshape(B, NX)
    # xT[p, t*B + b] = xf[b, t*128 + p]
    xT = np.ascontiguousarray(
        xf.T.reshape(KT, 128, B).transpose(1, 0, 2)
    ).reshape(128, XW).astype(X_NP)
    xTg = np.ascontiguousarray(np.tile(xT, (1, GROUP)))

    bins = np.asarray(binary)[0].astype(np.uint8)        # [8, 96, 768]
    bits = np.unpackbits(bins[:, :, :, None], axis=3)    # [..., p] = bit (7-p)
    sgn = bits.astype(np.float32) * 2.0 - 1.0            # [8k, 96m, 768f, 8p]
    sc = np.asarray(scale, dtype=np.float32)[0]          # [8, 768]
    W = np.einsum("kmfp,kf->mpf", sgn, sc).reshape(NX, NF)
    Wr = W.reshape(KT, 128, NCORES, FS)                  # [t, p, j, f]

    in_maps = []
    for j in range(NCORES):
        wj = np.ascontiguousarray(Wr[:, :, j, :].transpose(1, 0, 2)).reshape(
            128, WC
        ).astype(ml_dtypes.bfloat16)
        in_maps.append({"x": xTg, "w": np.ascontiguousarray(np.tile(wj, (1, GROUP)))})
    return in_maps


def kernel(x, scale, binary, bias, _trace=False):
    from concourse.bass_utils import run_bass_kernel_spmd

    if "nc" not in _CACHE:
        _CACHE["nc"] = _build_program()
    nc = _CACHE["nc"]

    in_maps = _prep_inputs(x, binary, scale)
    res = run_bass_kernel_spmd(nc, in_maps, core_ids=list(range(NCORES)), trace=_trace)
    _CACHE["last_result"] = res

    outT = np.concatenate(
        [np.asarray(res.results[j]["out"])[:, 0:B].astype(np.float32)
         for j in range(NCORES)],
        axis=0,
    )  # [768, 256]
    out = outT.T + np.asarray(bias, dtype=np.float32)[None, :]
    return out.reshape(4, 64, NF).astype(np.float32)
